# revision 22
# baseline (speedup 1.0000x reference)
"""Causal self-attention (B=4, T=2048, E=1024, H=16, D=64) on 8 TRN2 NeuronCores.

Sharding: core = b*2 + g  (data parallel over batch b in 0..3, tensor parallel
over head-halves g in 0..1; 8 local heads per core, column-split QKV /
row-split out projection). Host sums the two partial out-projections per batch
and adds b_out.

Device kernel (per core). All matmuls run with fp16 operands (1 cycle/row on
the PE) accumulating in fp32 PSUM; activations/weights are pre-cast to fp16 on
the host so they DMA straight into their SBUF tiles:
  - qT/kT [128 = 2 heads x 64, T] per head-pair; v' [T, 8 x (64 v-dims + ones
    col)]; the ones column makes the attn@v matmul emit softmax denominators.
  - transposed-scores attention per (pair, t-block of 512): scoresT[s,t]
    chunks via row-tiled K=64 matmul pairs into a 2-bank PSUM tile, one exp
    per chunk on ACT (both heads, scale=1/8 folded in), causal diagonal via
    in-place [128,128] triangle multiplies on DVE, av accumulated over
    s-chunks with causal width narrowing.
  - PE kept dense (HAM warm) while ACT grinds exps: remaining qkv-projection
    groups and out-projection groups are fed through the PSUM slot freed by
    each block's normalization.
  - normalization off the PE path: denominator rows bounce through DRAM and
    partition-broadcast back by DMA, reciprocal'd on DVE, multiplied into the
    fp16 attention output.
"""
import numpy as np

B, T, E, H, D = 4, 2048, 1024, 16, 64
HL = H // 2           # local heads per core (8)
NP = HL // 2          # head pairs per core (4)
EL = HL * D           # local attn-out width (512)
VW = HL * (D + 1)     # v' width with ones columns (520)
NCORES = 8
NB = T // 512         # t-blocks (4)
NC = T // 128         # s-chunks (16)
NE = E // 128         # e-chunks (8)

_cache = {}


def _build_nc():
    import concourse.bacc as bacc
    import concourse.mybir as mybir
    from concourse.tile import TileContext

    F32 = mybir.dt.float32
    F16 = mybir.dt.float16
    EXP = mybir.ActivationFunctionType.Exp

    nc = bacc.Bacc(None, target_bir_lowering=False)
    xT = nc.dram_tensor("xT", [E, T], F16, kind="ExternalInput")
    wqk = nc.dram_tensor("wqk", [2 * NP, 128, NE, 128], F16, kind="ExternalInput")
    wv2d = nc.dram_tensor("wv2d", [2, 128, NE, VW // 2], F16, kind="ExternalInput")
    wo = nc.dram_tensor("wo", [EL, E], F16, kind="ExternalInput")
    rowsd = nc.dram_tensor("rowsd", [1, VW], F16, kind="ExternalInput")   # bv2
    bcold = nc.dram_tensor("bcold", [128, 2 * NP], F32, kind="ExternalInput")
    trid = nc.dram_tensor("trid", [128, 128], F32, kind="ExternalInput")
    y = nc.dram_tensor("y", [T, E], F32, kind="ExternalOutput")

    with TileContext(nc) as tc:
        with (
            tc.tile_pool(name="const", bufs=1) as cpool,
            tc.tile_pool(name="p_keep", bufs=1) as keep,
            tc.tile_pool(name="p_st", bufs=2) as st,
        ):
            # ---- long-lived fp16 tensors, DMA'd directly (priority order) ----
            HALF = VW // 2  # 260
            xt = [keep.tile([128, T], F16, name=f"xt{e}", tag=f"xt{e}") for e in range(NE)]
            for e in range(NE):
                nc.sync.dma_start(out=xt[e], in_=xT[e * 128:(e + 1) * 128, :])
            wr = {}
            for p in range(NP):
                for i, nm in enumerate(("q", "k")):
                    wr[(p, nm)] = keep.tile([128, NE, 128], F16, name=f"w{nm}{p}", tag=f"w{nm}{p}")
            wv_r = [keep.tile([128, NE, HALF], F16, name=f"wv{h_}", tag=f"wv{h_}")
                    for h_ in range(2)]
            for i, nm in enumerate(("q", "k")):
                nc.sync.dma_start(out=wr[(0, nm)], in_=wqk[i])
            nc.sync.dma_start(out=wv_r[0], in_=wv2d[0])
            # ---- constants ----
            tri_sb = cpool.tile([128, 128], F32, name="tri_sb")
            nc.sync.dma_start(out=tri_sb, in_=trid[:, :])
            bcol = cpool.tile([128, 2 * NP], F32, name="bcol")
            nc.sync.dma_start(out=bcol, in_=bcold[:, :])
            ones_r = cpool.tile([1, 512], F16, name="ones_r")
            nc.vector.memset(ones_r, 1.0)
            bv_r = cpool.tile([1, VW], F16, name="bv_r")
            nc.sync.dma_start(out=bv_r, in_=rowsd[:, :])
            # preload the ACT exp table during the lead-in
            warm = cpool.tile([1, 16], F32, name="warm")
            nc.scalar.activation(warm, tri_sb[0:1, 0:16], EXP, scale=0.125)
            # remaining weights
            qt = [keep.tile([128, T], F16, name=f"qt{p}", tag=f"qt{p}") for p in range(NP)]
            kt = [keep.tile([128, T], F16, name=f"kt{p}", tag=f"kt{p}") for p in range(NP)]
            vt = [keep.tile([128, VW], F16, name=f"vt{t_}", tag=f"vt{t_}") for t_ in range(NC)]
            ao = [keep.tile([128, T], F16, name=f"ao{p}", tag=f"ao{p}") for p in range(NP)]
            wo_r = keep.tile([128, NP, E], F16, name="wo_r")
            for p in range(NP):
                for i, nm in enumerate(("q", "k")):
                    if p > 0:
                        nc.sync.dma_start(out=wr[(p, nm)], in_=wqk[2 * p + i])
            nc.sync.dma_start(out=wv_r[1], in_=wv2d[1])
            for p in range(NP):
                nc.sync.dma_start(out=wo_r[:, p, :], in_=wo[p * 128:(p + 1) * 128, :])

            with (
                tc.tile_pool(name="p_att", bufs=4) as att,
                tc.tile_pool(name="p_nrm", bufs=2) as nrm,
                tc.tile_pool(name="p_dr", bufs=2, space="DRAM") as drp,
                tc.tile_pool(name="psA", bufs=2, space="PSUM") as psA,
            ):
                def emit_qk_group(p, nm, tb, on_act=False):
                    dst = qt[p] if nm == "q" else kt[p]
                    ps = psA.tile([128, 512], F32, name=f"ps{nm}_{p}_{tb}", tag="av")
                    for e in range(NE):
                        nc.tensor.matmul(
                            ps, wr[(p, nm)][:, e, :], xt[e][:, tb * 512:(tb + 1) * 512],
                            start=(e == 0), stop=(e == NE - 1),
                        )
                    col = 2 * p + (0 if nm == "q" else 1)
                    dsl = dst[:, tb * 512:(tb + 1) * 512]
                    if on_act:
                        nc.scalar.add(dsl, ps, bcol[:, col:col + 1])
                    else:
                        nc.vector.tensor_scalar_add(dsl, ps, bcol[:, col:col + 1])

                def emit_v_group(tt, h_, on_act=False):
                    ps = psA.tile([128, HALF], F32, name=f"psv_{tt}_{h_}", tag="av")
                    for e in range(NE):
                        nc.tensor.matmul(
                            ps, xt[e][:, tt * 128:(tt + 1) * 128], wv_r[h_][:, e, :],
                            start=(e == 0), stop=False,
                        )
                    nc.tensor.matmul(
                        ps, ones_r[:, 0:128], bv_r[:, h_ * HALF:(h_ + 1) * HALF],
                        start=False, stop=True,
                    )
                    dsl = vt[tt][:, h_ * HALF:(h_ + 1) * HALF]
                    if on_act:
                        nc.scalar.copy(dsl, ps)
                    else:
                        nc.vector.tensor_copy(dsl, ps)

                def emit_scores(p, tb, c):
                    j = c - 4 * tb
                    lo = 128 * j if j >= 0 else 0
                    sp = psA.tile([128, 1024], F32, name=f"s_{p}_{tb}_{c}", tag="s")
                    nc.tensor.matmul(
                        sp[:, lo:512], kt[p][0:64, c * 128:(c + 1) * 128],
                        qt[p][0:64, tb * 512 + lo:(tb + 1) * 512],
                        start=True, stop=True, tile_position=(0, 0),
                    )
                    nc.tensor.matmul(
                        sp[:, 512 + lo:1024], kt[p][64:128, c * 128:(c + 1) * 128],
                        qt[p][64:128, tb * 512 + lo:(tb + 1) * 512],
                        start=True, stop=True, tile_position=(64, 0),
                    )
                    return sp, lo, j

                def emit_expav(p, tb, c, sc, av0, av1, nch):
                    sp, lo, j = sc
                    ep = att.tile([128, 1024], F16, name=f"e_{p}_{tb}_{c}", tag="ep")
                    if j < 0:
                        nc.scalar.activation(ep, sp, EXP, scale=0.125)
                    else:
                        spv = sp[:, :].rearrange("q (h t) -> q h t", h=2)
                        epv = ep[:, :].rearrange("q (h t) -> q h t", h=2)
                        nc.scalar.activation(epv[:, :, lo:512], spv[:, :, lo:512],
                                             EXP, scale=0.125)
                        for h in range(2):
                            nc.vector.tensor_mul(
                                epv[:, h, lo:lo + 128], epv[:, h, lo:lo + 128], tri_sb)
                    for h, av in ((0, av0), (1, av1)):
                        vcol = 65 * (2 * p + h)
                        nc.tensor.matmul(
                            av[:, lo:512], vt[c][:, vcol:vcol + 65],
                            ep[:, 512 * h + lo:512 * h + 512],
                            start=(c == 0), stop=(c == nch - 1),
                        )

                def emit_norm(p, tb, av0, av1):
                    dh = nrm.tile([65, 512], F32, name=f"dh_{p}_{tb}", tag="dh")
                    tmpu = nrm.tile([65, 512], F32, name=f"tu_{p}_{tb}", tag="tu")
                    ao_raw = nrm.tile([128, 512], F32, name=f"ar_{p}_{tb}", tag="ar")
                    nc.vector.tensor_copy(ao_raw[0:64, :], av0[0:64, :])
                    nc.vector.tensor_copy(dh[64:65, :], av0[64:65, :])
                    nc.vector.tensor_copy(tmpu, av1[0:65, :])
                    nc.gpsimd.dma_start(out=ao_raw[64:128, :], in_=tmpu[0:64, :])
                    dscr = drp.tile([2, 512], F32, name=f"ds_{p}_{tb}", tag="ds")
                    nc.gpsimd.dma_start(out=dscr[0:1, :], in_=dh[64:65, :])
                    nc.gpsimd.dma_start(out=dscr[1:2, :], in_=tmpu[64:65, :])
                    bc = nrm.tile([128, 512], F32, name=f"bc_{p}_{tb}", tag="bc")
                    nc.gpsimd.dma_start(
                        out=bc[0:64, :], in_=dscr[0:1, :].partition_broadcast(64))
                    nc.gpsimd.dma_start(
                        out=bc[64:128, :], in_=dscr[1:2, :].partition_broadcast(64))
                    rc = nrm.tile([128, 512], F32, name=f"rc_{p}_{tb}", tag="rc")
                    rcs = nrm.tile([128, 512], F32, name=f"rcs_{p}_{tb}", tag="rcs", bufs=1)
                    nc.vector.reciprocal_approx_accurate(out=rc, in_=bc, scratch=rcs)
                    nc.vector.tensor_mul(ao[p][:, tb * 512:(tb + 1) * 512], ao_raw, rc)

                def emit_proj_group(tt, eb, tag="av", on_act=False):
                    ps = psA.tile([128, 512], F32, name=f"py_{tt}_{eb}", tag=tag)
                    for pp in range(NP):
                        nc.tensor.matmul(
                            ps, ao[pp][:, tt * 128:(tt + 1) * 128],
                            wo_r[:, pp, eb * 512:(eb + 1) * 512],
                            start=(pp == 0), stop=(pp == NP - 1),
                        )
                    ys = st.tile([128, 512], F32, name=f"ys_{tt}_{eb}", tag="ys", bufs=3)
                    if on_act:
                        nc.scalar.copy(ys, ps)
                    else:
                        nc.vector.tensor_copy(ys, ps)
                    nc.sync.dma_start(
                        out=y[tt * 128:(tt + 1) * 128, eb * 512:(eb + 1) * 512], in_=ys)

                # ---- lead-in: qk pair 0 + v half 0 (pairs 0/1) ----
                for nm in ("q", "k"):
                    for tb in range(NB):
                        emit_qk_group(0, nm, tb, on_act=True)
                for tt in range(NC):
                    emit_v_group(tt, 0, on_act=True)
                for tt in range(NC):
                    emit_v_group(tt, 1, on_act=True)

                # window queue with deadlines: item = (deadline, kind, args);
                # deadline (p, tb) = must be emitted before block (p, tb) starts.
                def earlier(p_, tb_):
                    return (p_, tb_ - 1) if tb_ > 0 else (p_ - 1, NB - 1)

                wq_items = []
                for tb in range(NB):
                    for nm in ("q", "k"):
                        wq_items.append((earlier(1, tb), "qk", (1, nm, tb)))
                for pp in (2, 3):
                    for tb in range(NB):
                        for nm in ("q", "k"):
                            wq_items.append((earlier(pp, tb), "qk", (pp, nm, tb)))
                wq_items.sort(key=lambda it: it[0])
                NODL = (99, 99)

                def emit_item(kind, args):
                    if kind == "qk":
                        emit_qk_group(*args)
                    elif kind == "v":
                        emit_v_group(*args)
                    else:
                        emit_proj_group(*args)

                def pop_window(n):
                    for _ in range(n):
                        if wq_items:
                            _, kind, args = wq_items.pop(0)
                            emit_item(kind, args)

                def drain_due(key):
                    while wq_items and wq_items[0][0] <= key:
                        _, kind, args = wq_items.pop(0)
                        emit_item(kind, args)

                pending = None
                for p in range(NP):
                    for tb in range(NB):
                        drain_due((p, tb))
                        nch = 4 * (tb + 1)
                        avp = psA.tile([65, 1024], F32, name=f"avp_{p}_{tb}", tag="av")
                        av0 = avp[:, 0:512]
                        av1 = avp[:, 512:1024]
                        sc = emit_scores(p, tb, 0)
                        for c in range(nch):
                            sc_next = emit_scores(p, tb, c + 1) if c + 1 < nch else None
                            emit_expav(p, tb, c, sc, av0, av1, nch)
                            sc = sc_next
                            if c == min(3, nch - 1) and pending is not None:
                                np_, ntb_ = pending[0], pending[1]
                                emit_norm(*pending)
                                pending = None
                                if np_ == NP - 1:
                                    wq_items.extend(
                                        (NODL, "proj", (tt, eb))
                                        for tt in range(4 * ntb_, 4 * ntb_ + 4)
                                        for eb in range(2))
                                pop_window(3)
                        pending = (p, tb, av0, av1)
                emit_norm(*pending)
                wq_items.extend((NODL, "proj", (tt, eb))
                                for tt in range(12, 16) for eb in range(2))
                i = 0
                while wq_items:
                    _, kind, args = wq_items.pop(0)
                    if kind == "proj":
                        emit_proj_group(*args, tag=("s" if i % 2 else "av"), on_act=True)
                        i += 1
                    else:
                        emit_item(kind, args)

    nc.compile()
    return nc


def get_nc():
    if "nc" not in _cache:
        _cache["nc"] = _build_nc()
    return _cache["nc"]


def make_in_maps(x, w_qkv, b_qkv, w_out, b_out):
    """Per-core input dicts. Core = b*2 + g."""
    x = np.asarray(x, dtype=np.float32)
    w_qkv = np.asarray(w_qkv, dtype=np.float32)
    b_qkv = np.asarray(b_qkv, dtype=np.float32)
    w_out = np.asarray(w_out, dtype=np.float32)

    wq_full, wk_full, wv_full = w_qkv[:, 0:E], w_qkv[:, E:2 * E], w_qkv[:, 2 * E:3 * E]
    bq_full, bk_full, bv_full = b_qkv[0:E], b_qkv[E:2 * E], b_qkv[2 * E:3 * E]

    idx = np.arange(128)
    tri = (idx[:, None] <= idx[None, :]).astype(np.float32)  # tri[s,t]=1 iff s<=t

    in_maps = []
    for core in range(NCORES):
        b, g = core // 2, core % 2
        h0 = g * HL
        cols = slice(h0 * D, (h0 + HL) * D)
        wq_l = wq_full[:, cols]
        wk_l = wk_full[:, cols]
        wv_l = wv_full[:, cols]
        bq_l = bq_full[cols]
        bk_l = bk_full[cols]
        bv_l = bv_full[cols]

        wqk_s = np.empty((2 * NP, 128, NE, 128), dtype=np.float16)
        for p in range(NP):
            wqk_s[2 * p] = wq_l[:, p * 128:(p + 1) * 128].reshape(NE, 128, 128).transpose(1, 0, 2)
            wqk_s[2 * p + 1] = wk_l[:, p * 128:(p + 1) * 128].reshape(NE, 128, 128).transpose(1, 0, 2)

        wv2 = np.zeros((E, VW), dtype=np.float16)
        bv2 = np.zeros((1, VW), dtype=np.float16)
        for h in range(HL):
            wv2[:, h * 65:h * 65 + 64] = wv_l[:, h * 64:(h + 1) * 64].astype(np.float16)
            bv2[0, h * 65:h * 65 + 64] = bv_l[h * 64:(h + 1) * 64].astype(np.float16)
            bv2[0, h * 65 + 64] = 1.0

        bcol = np.zeros((128, 2 * NP), dtype=np.float32)
        for p in range(NP):
            bcol[:, 2 * p] = bq_l[p * 128:(p + 1) * 128]
            bcol[:, 2 * p + 1] = bk_l[p * 128:(p + 1) * 128]

        wv2d = wv2.reshape(NE, 128, 2, VW // 2).transpose(2, 1, 0, 3)
        in_maps.append({
            "xT": np.ascontiguousarray(x[b].T.astype(np.float16)),
            "wqk": np.ascontiguousarray(wqk_s),
            "wv2d": np.ascontiguousarray(wv2d),
            "wo": np.ascontiguousarray(w_out[g * EL:(g + 1) * EL, :]).astype(np.float16),
            "rowsd": bv2,
            "bcold": bcol,
            "trid": tri,
        })
    return in_maps


def gather_output(results, b_out):
    out = np.empty((B, T, E), dtype=np.float32)
    for b in range(B):
        out[b] = results[2 * b]["y"] + results[2 * b + 1]["y"] + b_out[None, :]
    return out


def kernel(x, w_qkv, b_qkv, w_out, b_out):
    from concourse.bass_utils import run_bass_kernel_spmd

    nc = get_nc()
    in_maps = make_in_maps(x, w_qkv, b_qkv, w_out, b_out)
    r = run_bass_kernel_spmd(nc, in_maps, core_ids=list(range(NCORES)))
    return gather_output(r.results, np.asarray(b_out, dtype=np.float32))


# revision 23
# speedup vs baseline: 1.1328x; 1.1328x over previous
"""Causal self-attention (B=4, T=2048, E=1024, H=16, D=64) on 8 TRN2 NeuronCores.

Sharding: core = b*2 + g  (data parallel over batch b in 0..3, tensor parallel
over head-halves g in 0..1; 8 local heads per core, column-split QKV /
row-split out projection). Host sums the two partial out-projections per batch
and adds b_out.

Device kernel (per core). All matmuls run with fp16 operands (1 cycle/row on
the PE) accumulating in fp32 PSUM; activations/weights are pre-cast to fp16 on
the host so they DMA straight into their SBUF tiles:
  - qT/kT [128 = 2 heads x 64, T] per head-pair; v' [T, 8 x (64 v-dims + ones
    col)]; the ones column makes the attn@v matmul emit softmax denominators.
  - transposed-scores attention per (pair, t-block of 512): scoresT[s,t]
    chunks via row-tiled K=64 matmul pairs into a 2-bank PSUM tile, one exp
    per chunk on ACT (both heads, scale=1/8 folded in), causal diagonal via
    in-place [128,128] triangle multiplies on DVE, av accumulated over
    s-chunks with causal width narrowing.
  - PE kept dense (HAM warm) while ACT grinds exps: remaining qkv-projection
    groups and out-projection groups are fed through the PSUM slot freed by
    each block's normalization.
  - normalization off the PE path: denominator rows bounce through DRAM and
    partition-broadcast back by DMA, reciprocal'd on DVE, multiplied into the
    fp16 attention output.
"""
import numpy as np

B, T, E, H, D = 4, 2048, 1024, 16, 64
HL = H // 2           # local heads per core (8)
NP = HL // 2          # head pairs per core (4)
EL = HL * D           # local attn-out width (512)
VW = HL * (D + 1)     # v' width with ones columns (520)
NCORES = 8
NB = T // 512         # t-blocks (4)
NC = T // 128         # s-chunks (16)
NE = E // 128         # e-chunks (8)

_cache = {}


def _build_nc():
    import concourse.bacc as bacc
    import concourse.mybir as mybir
    from concourse.tile import TileContext

    F32 = mybir.dt.float32
    F16 = mybir.dt.float16
    EXP = mybir.ActivationFunctionType.Exp

    nc = bacc.Bacc(None, target_bir_lowering=False)
    xT = nc.dram_tensor("xT", [E, T], F16, kind="ExternalInput")
    wqk = nc.dram_tensor("wqk", [2 * NP, 128, NE, 128], F16, kind="ExternalInput")
    wv2d = nc.dram_tensor("wv2d", [2, 128, NE, VW // 2], F16, kind="ExternalInput")
    wo = nc.dram_tensor("wo", [EL, E], F16, kind="ExternalInput")
    rowsd = nc.dram_tensor("rowsd", [1, VW], F16, kind="ExternalInput")   # bv2
    bcold = nc.dram_tensor("bcold", [128, 2 * NP], F32, kind="ExternalInput")
    trid = nc.dram_tensor("trid", [128, 128], F32, kind="ExternalInput")
    y = nc.dram_tensor("y", [T, E], F32, kind="ExternalOutput")

    with TileContext(nc) as tc:
        with (
            tc.tile_pool(name="const", bufs=1) as cpool,
            tc.tile_pool(name="p_keep", bufs=1) as keep,
            tc.tile_pool(name="p_st", bufs=2) as st,
        ):
            # ---- long-lived fp16 tensors, DMA'd directly (priority order) ----
            HALF = VW // 2  # 260
            xt = [keep.tile([128, T], F16, name=f"xt{e}", tag=f"xt{e}") for e in range(NE)]
            for e in range(NE):
                nc.sync.dma_start(out=xt[e], in_=xT[e * 128:(e + 1) * 128, :])
            wr = {}
            for p in range(NP):
                for i, nm in enumerate(("q", "k")):
                    wr[(p, nm)] = keep.tile([128, NE, 128], F16, name=f"w{nm}{p}", tag=f"w{nm}{p}")
            wv_r = [keep.tile([128, NE, HALF], F16, name=f"wv{h_}", tag=f"wv{h_}")
                    for h_ in range(2)]
            for i, nm in enumerate(("q", "k")):
                nc.sync.dma_start(out=wr[(0, nm)], in_=wqk[i])
            nc.sync.dma_start(out=wv_r[0], in_=wv2d[0])
            # ---- constants ----
            tri_sb = cpool.tile([128, 128], F32, name="tri_sb")
            nc.sync.dma_start(out=tri_sb, in_=trid[:, :])
            bcol = cpool.tile([128, 2 * NP], F32, name="bcol")
            nc.sync.dma_start(out=bcol, in_=bcold[:, :])
            ones_r = cpool.tile([1, 512], F16, name="ones_r")
            nc.vector.memset(ones_r, 1.0)
            bv_r = cpool.tile([1, VW], F16, name="bv_r")
            nc.sync.dma_start(out=bv_r, in_=rowsd[:, :])
            # preload the ACT exp table during the lead-in
            warm = cpool.tile([1, 16], F32, name="warm")
            nc.scalar.activation(warm, tri_sb[0:1, 0:16], EXP, scale=0.125)
            # remaining weights
            qt = [keep.tile([128, T], F16, name=f"qt{p}", tag=f"qt{p}") for p in range(NP)]
            kt = [keep.tile([128, T], F16, name=f"kt{p}", tag=f"kt{p}") for p in range(NP)]
            vt = [keep.tile([128, VW], F16, name=f"vt{t_}", tag=f"vt{t_}") for t_ in range(NC)]
            ao = [keep.tile([128, T], F16, name=f"ao{p}", tag=f"ao{p}") for p in range(NP)]
            wo_r = keep.tile([128, NP, E], F16, name="wo_r")
            for p in range(NP):
                for i, nm in enumerate(("q", "k")):
                    if p > 0:
                        nc.sync.dma_start(out=wr[(p, nm)], in_=wqk[2 * p + i])
            nc.sync.dma_start(out=wv_r[1], in_=wv2d[1])
            for p in range(NP):
                nc.sync.dma_start(out=wo_r[:, p, :], in_=wo[p * 128:(p + 1) * 128, :])

            with (
                tc.tile_pool(name="p_att", bufs=3) as att,
                tc.tile_pool(name="p_nrm", bufs=2) as nrm,
                tc.tile_pool(name="p_dr", bufs=2, space="DRAM") as drp,
                tc.tile_pool(name="psA", bufs=2, space="PSUM") as psA,
            ):
                def emit_qk_group(p, nm, tb, on_act=False):
                    dst = qt[p] if nm == "q" else kt[p]
                    ps = psA.tile([128, 512], F32, name=f"ps{nm}_{p}_{tb}", tag="av")
                    for e in range(NE):
                        nc.tensor.matmul(
                            ps, wr[(p, nm)][:, e, :], xt[e][:, tb * 512:(tb + 1) * 512],
                            start=(e == 0), stop=(e == NE - 1),
                        )
                    col = 2 * p + (0 if nm == "q" else 1)
                    dsl = dst[:, tb * 512:(tb + 1) * 512]
                    if on_act:
                        nc.scalar.add(dsl, ps, bcol[:, col:col + 1])
                    else:
                        nc.vector.tensor_scalar_add(dsl, ps, bcol[:, col:col + 1])

                def emit_v_group(tt, h_, on_act=False):
                    ps = psA.tile([128, HALF], F32, name=f"psv_{tt}_{h_}", tag="av")
                    for e in range(NE):
                        nc.tensor.matmul(
                            ps, xt[e][:, tt * 128:(tt + 1) * 128], wv_r[h_][:, e, :],
                            start=(e == 0), stop=False,
                        )
                    nc.tensor.matmul(
                        ps, ones_r[:, 0:128], bv_r[:, h_ * HALF:(h_ + 1) * HALF],
                        start=False, stop=True,
                    )
                    dsl = vt[tt][:, h_ * HALF:(h_ + 1) * HALF]
                    if on_act:
                        nc.scalar.copy(dsl, ps)
                    else:
                        nc.vector.tensor_copy(dsl, ps)

                def emit_scores(p, tb, c):
                    j = c - 4 * tb
                    lo = 128 * j if j >= 0 else 0
                    sp = psA.tile([128, 1024], F32, name=f"s_{p}_{tb}_{c}", tag="s")
                    nc.tensor.matmul(
                        sp[:, lo:512], kt[p][0:64, c * 128:(c + 1) * 128],
                        qt[p][0:64, tb * 512 + lo:(tb + 1) * 512],
                        start=True, stop=True, tile_position=(0, 0),
                    )
                    nc.tensor.matmul(
                        sp[:, 512 + lo:1024], kt[p][64:128, c * 128:(c + 1) * 128],
                        qt[p][64:128, tb * 512 + lo:(tb + 1) * 512],
                        start=True, stop=True, tile_position=(64, 0),
                    )
                    return sp, lo, j

                def emit_expav(p, tb, c, sc, av0, av1, nch):
                    sp, lo, j = sc
                    ep = att.tile([128, 1024], F16, name=f"e_{p}_{tb}_{c}", tag="ep")
                    if j < 0:
                        nc.scalar.activation(ep, sp, EXP, scale=0.125)
                    else:
                        spv = sp[:, :].rearrange("q (h t) -> q h t", h=2)
                        epv = ep[:, :].rearrange("q (h t) -> q h t", h=2)
                        nc.scalar.activation(epv[:, :, lo:512], spv[:, :, lo:512],
                                             EXP, scale=0.125)
                        for h in range(2):
                            nc.vector.tensor_mul(
                                epv[:, h, lo:lo + 128], epv[:, h, lo:lo + 128], tri_sb)
                    for h, av in ((0, av0), (1, av1)):
                        vcol = 65 * (2 * p + h)
                        nc.tensor.matmul(
                            av[:, lo:512], vt[c][:, vcol:vcol + 65],
                            ep[:, 512 * h + lo:512 * h + 512],
                            start=(c == 0), stop=(c == nch - 1),
                        )

                def emit_norm(p, tb, av0, av1):
                    dh = nrm.tile([65, 512], F32, name=f"dh_{p}_{tb}", tag="dh")
                    tmpu = nrm.tile([65, 512], F32, name=f"tu_{p}_{tb}", tag="tu")
                    ao_raw = nrm.tile([128, 512], F32, name=f"ar_{p}_{tb}", tag="ar")
                    nc.vector.tensor_copy(ao_raw[0:64, :], av0[0:64, :])
                    nc.vector.tensor_copy(dh[64:65, :], av0[64:65, :])
                    nc.vector.tensor_copy(tmpu, av1[0:65, :])
                    nc.gpsimd.dma_start(out=ao_raw[64:128, :], in_=tmpu[0:64, :])
                    dscr = drp.tile([2, 512], F32, name=f"ds_{p}_{tb}", tag="ds")
                    nc.gpsimd.dma_start(out=dscr[0:1, :], in_=dh[64:65, :])
                    nc.gpsimd.dma_start(out=dscr[1:2, :], in_=tmpu[64:65, :])
                    bc = nrm.tile([128, 512], F32, name=f"bc_{p}_{tb}", tag="bc")
                    nc.gpsimd.dma_start(
                        out=bc[0:64, :], in_=dscr[0:1, :].partition_broadcast(64))
                    nc.gpsimd.dma_start(
                        out=bc[64:128, :], in_=dscr[1:2, :].partition_broadcast(64))
                    rc = nrm.tile([128, 512], F32, name=f"rc_{p}_{tb}", tag="rc")
                    rcs = nrm.tile([128, 512], F32, name=f"rcs_{p}_{tb}", tag="rcs", bufs=1)
                    nc.vector.reciprocal_approx_accurate(out=rc, in_=bc, scratch=rcs)
                    nc.vector.tensor_mul(ao[p][:, tb * 512:(tb + 1) * 512], ao_raw, rc)

                def emit_proj_group(tt, eb, tag="av", on_act=False):
                    ps = psA.tile([128, 512], F32, name=f"py_{tt}_{eb}", tag=tag)
                    for pp in range(NP):
                        nc.tensor.matmul(
                            ps, ao[pp][:, tt * 128:(tt + 1) * 128],
                            wo_r[:, pp, eb * 512:(eb + 1) * 512],
                            start=(pp == 0), stop=(pp == NP - 1),
                        )
                    ys = st.tile([128, 512], F32, name=f"ys_{tt}_{eb}", tag="ys", bufs=3)
                    if on_act:
                        nc.scalar.copy(ys, ps)
                    else:
                        nc.vector.tensor_copy(ys, ps)
                    nc.sync.dma_start(
                        out=y[tt * 128:(tt + 1) * 128, eb * 512:(eb + 1) * 512], in_=ys)

                # ---- lead-in: qk pair 0 + v half 0 (pairs 0/1) ----
                for nm in ("q", "k"):
                    for tb in range(NB):
                        emit_qk_group(0, nm, tb, on_act=True)
                for tt in range(NC):
                    emit_v_group(tt, 0, on_act=True)
                for tt in range(NC):
                    emit_v_group(tt, 1, on_act=True)

                # window queue with deadlines: item = (deadline, kind, args);
                # deadline (p, tb) = must be emitted before block (p, tb) starts.
                def earlier(p_, tb_):
                    return (p_, tb_ - 1) if tb_ > 0 else (p_ - 1, NB - 1)

                wq_items = []
                for tb in range(NB):
                    for nm in ("q", "k"):
                        wq_items.append((earlier(1, tb), "qk", (1, nm, tb)))
                for pp in (2, 3):
                    for tb in range(NB):
                        for nm in ("q", "k"):
                            wq_items.append((earlier(pp, tb), "qk", (pp, nm, tb)))
                wq_items.sort(key=lambda it: it[0])
                NODL = (99, 99)

                def emit_item(kind, args):
                    if kind == "qk":
                        emit_qk_group(*args)
                    elif kind == "v":
                        emit_v_group(*args)
                    else:
                        emit_proj_group(*args)

                def pop_window(n):
                    for _ in range(n):
                        if wq_items:
                            _, kind, args = wq_items.pop(0)
                            emit_item(kind, args)

                def drain_due(key):
                    while wq_items and wq_items[0][0] <= key:
                        _, kind, args = wq_items.pop(0)
                        emit_item(kind, args)

                pending = None
                for p in range(NP):
                    for tb in range(NB):
                        drain_due((p, tb))
                        nch = 4 * (tb + 1)
                        avp = psA.tile([65, 1024], F32, name=f"avp_{p}_{tb}", tag="av")
                        av0 = avp[:, 0:512]
                        av1 = avp[:, 512:1024]
                        sc = emit_scores(p, tb, 0)
                        for c in range(nch):
                            sc_next = emit_scores(p, tb, c + 1) if c + 1 < nch else None
                            emit_expav(p, tb, c, sc, av0, av1, nch)
                            sc = sc_next
                            if c == min(3, nch - 1) and pending is not None:
                                np_, ntb_ = pending[0], pending[1]
                                emit_norm(*pending)
                                pending = None
                                if np_ == NP - 1:
                                    wq_items.extend(
                                        (NODL, "proj", (tt, eb))
                                        for tt in range(4 * ntb_, 4 * ntb_ + 4)
                                        for eb in range(2))
                                pop_window(3)
                        pending = (p, tb, av0, av1)
                emit_norm(*pending)
                wq_items.extend((NODL, "proj", (tt, eb))
                                for tt in range(12, 16) for eb in range(2))
                i = 0
                while wq_items:
                    _, kind, args = wq_items.pop(0)
                    if kind == "proj":
                        emit_proj_group(*args, tag=("s" if i % 2 else "av"), on_act=True)
                        i += 1
                    else:
                        emit_item(kind, args)

    nc.compile()
    return nc


def get_nc():
    if "nc" not in _cache:
        _cache["nc"] = _build_nc()
    return _cache["nc"]


def make_in_maps(x, w_qkv, b_qkv, w_out, b_out):
    """Per-core input dicts. Core = b*2 + g."""
    x = np.asarray(x, dtype=np.float32)
    w_qkv = np.asarray(w_qkv, dtype=np.float32)
    b_qkv = np.asarray(b_qkv, dtype=np.float32)
    w_out = np.asarray(w_out, dtype=np.float32)

    wq_full, wk_full, wv_full = w_qkv[:, 0:E], w_qkv[:, E:2 * E], w_qkv[:, 2 * E:3 * E]
    bq_full, bk_full, bv_full = b_qkv[0:E], b_qkv[E:2 * E], b_qkv[2 * E:3 * E]

    idx = np.arange(128)
    tri = (idx[:, None] <= idx[None, :]).astype(np.float32)  # tri[s,t]=1 iff s<=t

    in_maps = []
    for core in range(NCORES):
        b, g = core // 2, core % 2
        h0 = g * HL
        cols = slice(h0 * D, (h0 + HL) * D)
        wq_l = wq_full[:, cols]
        wk_l = wk_full[:, cols]
        wv_l = wv_full[:, cols]
        bq_l = bq_full[cols]
        bk_l = bk_full[cols]
        bv_l = bv_full[cols]

        wqk_s = np.empty((2 * NP, 128, NE, 128), dtype=np.float16)
        for p in range(NP):
            wqk_s[2 * p] = wq_l[:, p * 128:(p + 1) * 128].reshape(NE, 128, 128).transpose(1, 0, 2)
            wqk_s[2 * p + 1] = wk_l[:, p * 128:(p + 1) * 128].reshape(NE, 128, 128).transpose(1, 0, 2)

        wv2 = np.zeros((E, VW), dtype=np.float16)
        bv2 = np.zeros((1, VW), dtype=np.float16)
        for h in range(HL):
            wv2[:, h * 65:h * 65 + 64] = wv_l[:, h * 64:(h + 1) * 64].astype(np.float16)
            bv2[0, h * 65:h * 65 + 64] = bv_l[h * 64:(h + 1) * 64].astype(np.float16)
            bv2[0, h * 65 + 64] = 1.0

        bcol = np.zeros((128, 2 * NP), dtype=np.float32)
        for p in range(NP):
            bcol[:, 2 * p] = bq_l[p * 128:(p + 1) * 128]
            bcol[:, 2 * p + 1] = bk_l[p * 128:(p + 1) * 128]

        wv2d = wv2.reshape(NE, 128, 2, VW // 2).transpose(2, 1, 0, 3)
        in_maps.append({
            "xT": np.ascontiguousarray(x[b].T.astype(np.float16)),
            "wqk": np.ascontiguousarray(wqk_s),
            "wv2d": np.ascontiguousarray(wv2d),
            "wo": np.ascontiguousarray(w_out[g * EL:(g + 1) * EL, :]).astype(np.float16),
            "rowsd": bv2,
            "bcold": bcol,
            "trid": tri,
        })
    return in_maps


def gather_output(results, b_out):
    out = np.empty((B, T, E), dtype=np.float32)
    for b in range(B):
        out[b] = results[2 * b]["y"] + results[2 * b + 1]["y"] + b_out[None, :]
    return out


def kernel(x, w_qkv, b_qkv, w_out, b_out):
    from concourse.bass_utils import run_bass_kernel_spmd

    nc = get_nc()
    in_maps = make_in_maps(x, w_qkv, b_qkv, w_out, b_out)
    r = run_bass_kernel_spmd(nc, in_maps, core_ids=list(range(NCORES)))
    return gather_output(r.results, np.asarray(b_out, dtype=np.float32))


# revision 24
# speedup vs baseline: 1.1457x; 1.0114x over previous
"""Causal self-attention (B=4, T=2048, E=1024, H=16, D=64) on 8 TRN2 NeuronCores.

Sharding: core = b*2 + g  (data parallel over batch b in 0..3, tensor parallel
over head-halves g in 0..1; 8 local heads per core, column-split QKV /
row-split out projection). Host sums the two partial out-projections per batch
and adds b_out.

Device kernel (per core). All matmuls run with fp16 operands (1 cycle/row on
the PE) accumulating in fp32 PSUM; activations/weights are pre-cast to fp16 on
the host so they DMA straight into their SBUF tiles:
  - qT/kT [128 = 2 heads x 64, T] per head-pair; v' [T, 8 x (64 v-dims + ones
    col)]; the ones column makes the attn@v matmul emit softmax denominators.
  - transposed-scores attention per (pair, t-block of 512): scoresT[s,t]
    chunks via row-tiled K=64 matmul pairs into a 2-bank PSUM tile, one exp
    per chunk on ACT (both heads, scale=1/8 folded in), causal diagonal via
    in-place [128,128] triangle multiplies on DVE, av accumulated over
    s-chunks with causal width narrowing.
  - PE kept dense (HAM warm) while ACT grinds exps: remaining qkv-projection
    groups and out-projection groups are fed through the PSUM slot freed by
    each block's normalization.
  - normalization off the PE path: denominator rows bounce through DRAM and
    partition-broadcast back by DMA, reciprocal'd on DVE, multiplied into the
    fp16 attention output.
"""
import numpy as np

B, T, E, H, D = 4, 2048, 1024, 16, 64
HL = H // 2           # local heads per core (8)
NP = HL // 2          # head pairs per core (4)
EL = HL * D           # local attn-out width (512)
VW = HL * (D + 1)     # v' width with ones columns (520)
NCORES = 8
NB = T // 512         # t-blocks (4)
NC = T // 128         # s-chunks (16)
NE = E // 128         # e-chunks (8)

_cache = {}


def _build_nc():
    import concourse.bacc as bacc
    import concourse.mybir as mybir
    from concourse.tile import TileContext

    F32 = mybir.dt.float32
    F16 = mybir.dt.float16
    EXP = mybir.ActivationFunctionType.Exp

    nc = bacc.Bacc(None, target_bir_lowering=False)
    xT = nc.dram_tensor("xT", [E, T], F16, kind="ExternalInput")
    wqk = nc.dram_tensor("wqk", [2 * NP, 128, NE, 128], F16, kind="ExternalInput")
    wv2d = nc.dram_tensor("wv2d", [2, 128, NE, VW // 2], F16, kind="ExternalInput")
    wo = nc.dram_tensor("wo", [EL, E], F16, kind="ExternalInput")
    rowsd = nc.dram_tensor("rowsd", [1, VW], F16, kind="ExternalInput")   # bv2
    bcold = nc.dram_tensor("bcold", [128, 2 * NP], F32, kind="ExternalInput")
    trid = nc.dram_tensor("trid", [128, 128], F32, kind="ExternalInput")
    y = nc.dram_tensor("y", [T, E], F32, kind="ExternalOutput")

    with TileContext(nc) as tc:
        with (
            tc.tile_pool(name="const", bufs=1) as cpool,
            tc.tile_pool(name="p_keep", bufs=1) as keep,
            tc.tile_pool(name="p_st", bufs=2) as st,
        ):
            # ---- long-lived fp16 tensors, DMA'd directly (priority order) ----
            HALF = VW // 2  # 260
            xt = [keep.tile([128, T], F16, name=f"xt{e}", tag=f"xt{e}") for e in range(NE)]
            for e in range(NE):
                nc.sync.dma_start(out=xt[e], in_=xT[e * 128:(e + 1) * 128, :])
            wr = {}
            for p in range(NP):
                for i, nm in enumerate(("q", "k")):
                    wr[(p, nm)] = keep.tile([128, NE, 128], F16, name=f"w{nm}{p}", tag=f"w{nm}{p}")
            wv_r = [keep.tile([128, NE, HALF], F16, name=f"wv{h_}", tag=f"wv{h_}")
                    for h_ in range(2)]
            for i, nm in enumerate(("q", "k")):
                nc.sync.dma_start(out=wr[(0, nm)], in_=wqk[i])
            nc.sync.dma_start(out=wv_r[0], in_=wv2d[0])
            # ---- constants ----
            tri_sb = cpool.tile([128, 128], F32, name="tri_sb")
            nc.sync.dma_start(out=tri_sb, in_=trid[:, :])
            bcol = cpool.tile([128, 2 * NP], F32, name="bcol")
            nc.sync.dma_start(out=bcol, in_=bcold[:, :])
            ones_r = cpool.tile([1, 512], F16, name="ones_r")
            nc.vector.memset(ones_r, 1.0)
            bv_r = cpool.tile([1, VW], F16, name="bv_r")
            nc.sync.dma_start(out=bv_r, in_=rowsd[:, :])
            # preload the ACT exp table during the lead-in
            warm = cpool.tile([1, 16], F32, name="warm")
            nc.scalar.activation(warm, tri_sb[0:1, 0:16], EXP, scale=0.125)
            # remaining weights
            qt = [keep.tile([128, T], F16, name=f"qt{p}", tag=f"qt{p}") for p in range(NP)]
            kt = [keep.tile([128, T], F16, name=f"kt{p}", tag=f"kt{p}") for p in range(NP)]
            vt = [keep.tile([128, VW], F16, name=f"vt{t_}", tag=f"vt{t_}") for t_ in range(NC)]
            ao = [keep.tile([128, T], F16, name=f"ao{p}", tag=f"ao{p}") for p in range(NP)]
            wo_r = keep.tile([128, NP, E], F16, name="wo_r")
            for p in range(NP):
                for i, nm in enumerate(("q", "k")):
                    if p > 0:
                        nc.sync.dma_start(out=wr[(p, nm)], in_=wqk[2 * p + i])
            nc.sync.dma_start(out=wv_r[1], in_=wv2d[1])
            for p in range(NP):
                nc.sync.dma_start(out=wo_r[:, p, :], in_=wo[p * 128:(p + 1) * 128, :])

            with (
                tc.tile_pool(name="p_att", bufs=3) as att,
                tc.tile_pool(name="p_nrm", bufs=2) as nrm,
                tc.tile_pool(name="p_dr", bufs=2, space="DRAM") as drp,
                tc.tile_pool(name="psA", bufs=2, space="PSUM") as psA,
            ):
                def emit_qk_group(p, nm, tb, on_act=False):
                    dst = qt[p] if nm == "q" else kt[p]
                    ps = psA.tile([128, 512], F32, name=f"ps{nm}_{p}_{tb}", tag="av")
                    for e in range(NE):
                        nc.tensor.matmul(
                            ps, wr[(p, nm)][:, e, :], xt[e][:, tb * 512:(tb + 1) * 512],
                            start=(e == 0), stop=(e == NE - 1),
                        )
                    col = 2 * p + (0 if nm == "q" else 1)
                    dsl = dst[:, tb * 512:(tb + 1) * 512]
                    if on_act:
                        nc.scalar.add(dsl, ps, bcol[:, col:col + 1])
                    else:
                        nc.vector.tensor_scalar_add(dsl, ps, bcol[:, col:col + 1])

                def emit_v_group(tt, h_, on_act=False):
                    ps = psA.tile([128, HALF], F32, name=f"psv_{tt}_{h_}", tag="av")
                    for e in range(NE):
                        nc.tensor.matmul(
                            ps, xt[e][:, tt * 128:(tt + 1) * 128], wv_r[h_][:, e, :],
                            start=(e == 0), stop=False,
                        )
                    nc.tensor.matmul(
                        ps, ones_r[:, 0:128], bv_r[:, h_ * HALF:(h_ + 1) * HALF],
                        start=False, stop=True,
                    )
                    dsl = vt[tt][:, h_ * HALF:(h_ + 1) * HALF]
                    if on_act:
                        nc.scalar.copy(dsl, ps)
                    else:
                        nc.vector.tensor_copy(dsl, ps)

                def emit_scores(p, tb, c):
                    j = c - 4 * tb
                    lo = 128 * j if j >= 0 else 0
                    sp = psA.tile([128, 1024], F32, name=f"s_{p}_{tb}_{c}", tag="s")
                    nc.tensor.matmul(
                        sp[:, lo:512], kt[p][0:64, c * 128:(c + 1) * 128],
                        qt[p][0:64, tb * 512 + lo:(tb + 1) * 512],
                        start=True, stop=True, tile_position=(0, 0),
                    )
                    nc.tensor.matmul(
                        sp[:, 512 + lo:1024], kt[p][64:128, c * 128:(c + 1) * 128],
                        qt[p][64:128, tb * 512 + lo:(tb + 1) * 512],
                        start=True, stop=True, tile_position=(64, 0),
                    )
                    return sp, lo, j

                def emit_expav(p, tb, c, sc, av0, av1, nch):
                    sp, lo, j = sc
                    ep = att.tile([128, 1024], F16, name=f"e_{p}_{tb}_{c}", tag="ep")
                    if j < 0:
                        nc.scalar.activation(ep, sp, EXP, scale=0.125)
                    else:
                        spv = sp[:, :].rearrange("q (h t) -> q h t", h=2)
                        epv = ep[:, :].rearrange("q (h t) -> q h t", h=2)
                        nc.scalar.activation(epv[:, :, lo:512], spv[:, :, lo:512],
                                             EXP, scale=0.125)
                        for h in range(2):
                            nc.vector.tensor_mul(
                                epv[:, h, lo:lo + 128], epv[:, h, lo:lo + 128], tri_sb)
                    for h, av in ((0, av0), (1, av1)):
                        vcol = 65 * (2 * p + h)
                        nc.tensor.matmul(
                            av[:, lo:512], vt[c][:, vcol:vcol + 65],
                            ep[:, 512 * h + lo:512 * h + 512],
                            start=(c == 0), stop=(c == nch - 1),
                        )

                def emit_norm(p, tb, av0, av1):
                    dh = nrm.tile([65, 512], F32, name=f"dh_{p}_{tb}", tag="dh")
                    tmpu = nrm.tile([65, 512], F32, name=f"tu_{p}_{tb}", tag="tu")
                    ao_raw = nrm.tile([128, 512], F32, name=f"ar_{p}_{tb}", tag="ar")
                    nc.vector.tensor_copy(ao_raw[0:64, :], av0[0:64, :])
                    nc.vector.tensor_copy(dh[64:65, :], av0[64:65, :])
                    nc.vector.tensor_copy(tmpu, av1[0:65, :])
                    nc.gpsimd.dma_start(out=ao_raw[64:128, :], in_=tmpu[0:64, :])
                    dscr = drp.tile([2, 512], F32, name=f"ds_{p}_{tb}", tag="ds")
                    nc.gpsimd.dma_start(out=dscr[0:1, :], in_=dh[64:65, :])
                    nc.gpsimd.dma_start(out=dscr[1:2, :], in_=tmpu[64:65, :])
                    bc = nrm.tile([128, 512], F32, name=f"bc_{p}_{tb}", tag="bc")
                    nc.gpsimd.dma_start(
                        out=bc[0:64, :], in_=dscr[0:1, :].partition_broadcast(64))
                    nc.gpsimd.dma_start(
                        out=bc[64:128, :], in_=dscr[1:2, :].partition_broadcast(64))
                    rc = nrm.tile([128, 512], F32, name=f"rc_{p}_{tb}", tag="rc")
                    rcs = nrm.tile([128, 512], F32, name=f"rcs_{p}_{tb}", tag="rcs", bufs=1)
                    nc.vector.reciprocal_approx_accurate(out=rc, in_=bc, scratch=rcs)
                    nc.vector.tensor_mul(ao[p][:, tb * 512:(tb + 1) * 512], ao_raw, rc)

                def emit_proj_group(tt, eb, tag="av", on_act=False):
                    ps = psA.tile([128, 512], F32, name=f"py_{tt}_{eb}", tag=tag)
                    for pp in range(NP):
                        nc.tensor.matmul(
                            ps, ao[pp][:, tt * 128:(tt + 1) * 128],
                            wo_r[:, pp, eb * 512:(eb + 1) * 512],
                            start=(pp == 0), stop=(pp == NP - 1),
                        )
                    ys = st.tile([128, 512], F32, name=f"ys_{tt}_{eb}", tag="ys", bufs=3)
                    if on_act:
                        nc.scalar.copy(ys, ps)
                    else:
                        nc.vector.tensor_copy(ys, ps)
                    nc.sync.dma_start(
                        out=y[tt * 128:(tt + 1) * 128, eb * 512:(eb + 1) * 512], in_=ys)

                # ---- lead-in: qk pair 0 + v half 0 (pairs 0/1) ----
                for nm in ("q", "k"):
                    for tb in range(NB):
                        emit_qk_group(0, nm, tb, on_act=True)
                for tt in range(NC):
                    emit_v_group(tt, 0, on_act=True)
                for tt in range(NC):
                    emit_v_group(tt, 1, on_act=True)

                # window queue with deadlines: item = (deadline, kind, args);
                # deadline (p, tb) = must be emitted before block (p, tb) starts.
                def earlier(p_, tb_):
                    return (p_, tb_ - 1) if tb_ > 0 else (p_ - 1, NB - 1)

                wq_items = []
                for tb in range(NB):
                    for nm in ("q", "k"):
                        wq_items.append((earlier(1, tb), "qk", (1, nm, tb)))
                for pp in (2, 3):
                    for tb in range(NB):
                        for nm in ("q", "k"):
                            wq_items.append((earlier(pp, tb), "qk", (pp, nm, tb)))
                wq_items.sort(key=lambda it: it[0])
                NODL = (99, 99)

                def emit_item(kind, args):
                    if kind == "qk":
                        emit_qk_group(*args)
                    elif kind == "v":
                        emit_v_group(*args)
                    else:
                        emit_proj_group(*args)

                def pop_window(n):
                    for _ in range(n):
                        if wq_items:
                            _, kind, args = wq_items.pop(0)
                            emit_item(kind, args)

                def drain_due(key):
                    while wq_items and wq_items[0][0] <= key:
                        _, kind, args = wq_items.pop(0)
                        emit_item(kind, args)

                pending = None
                blocks = [(p, tb) for p in range(NP) for tb in range(NB)]
                pre_sc = None
                for bi, (p, tb) in enumerate(blocks):
                    drain_due((p, tb))
                    nch = 4 * (tb + 1)
                    avp = psA.tile([65, 1024], F32, name=f"avp_{p}_{tb}", tag="av")
                    av0 = avp[:, 0:512]
                    av1 = avp[:, 512:1024]
                    sc = pre_sc if pre_sc is not None else emit_scores(p, tb, 0)
                    pre_sc = None
                    for c in range(nch):
                        if c + 1 < nch:
                            sc_next = emit_scores(p, tb, c + 1)
                        elif bi + 1 < len(blocks):
                            # cross-block lookahead: next block's first scores
                            pre_sc = emit_scores(blocks[bi + 1][0], blocks[bi + 1][1], 0)
                            sc_next = None
                        else:
                            sc_next = None
                        emit_expav(p, tb, c, sc, av0, av1, nch)
                        sc = sc_next
                        if c == min(3, nch - 1) and pending is not None:
                            np_, ntb_ = pending[0], pending[1]
                            emit_norm(*pending)
                            pending = None
                            if np_ == NP - 1:
                                wq_items.extend(
                                    (NODL, "proj", (tt, eb))
                                    for tt in range(4 * ntb_, 4 * ntb_ + 4)
                                    for eb in range(2))
                            pop_window(3)
                    pending = (p, tb, av0, av1)
                emit_norm(*pending)
                wq_items.extend((NODL, "proj", (tt, eb))
                                for tt in range(12, 16) for eb in range(2))
                i = 0
                while wq_items:
                    _, kind, args = wq_items.pop(0)
                    if kind == "proj":
                        emit_proj_group(*args, tag=("s" if i % 2 else "av"), on_act=True)
                        i += 1
                    else:
                        emit_item(kind, args)

    nc.compile()
    return nc


def get_nc():
    if "nc" not in _cache:
        _cache["nc"] = _build_nc()
    return _cache["nc"]


def make_in_maps(x, w_qkv, b_qkv, w_out, b_out):
    """Per-core input dicts. Core = b*2 + g."""
    x = np.asarray(x, dtype=np.float32)
    w_qkv = np.asarray(w_qkv, dtype=np.float32)
    b_qkv = np.asarray(b_qkv, dtype=np.float32)
    w_out = np.asarray(w_out, dtype=np.float32)

    wq_full, wk_full, wv_full = w_qkv[:, 0:E], w_qkv[:, E:2 * E], w_qkv[:, 2 * E:3 * E]
    bq_full, bk_full, bv_full = b_qkv[0:E], b_qkv[E:2 * E], b_qkv[2 * E:3 * E]

    idx = np.arange(128)
    tri = (idx[:, None] <= idx[None, :]).astype(np.float32)  # tri[s,t]=1 iff s<=t

    in_maps = []
    for core in range(NCORES):
        b, g = core // 2, core % 2
        h0 = g * HL
        cols = slice(h0 * D, (h0 + HL) * D)
        wq_l = wq_full[:, cols]
        wk_l = wk_full[:, cols]
        wv_l = wv_full[:, cols]
        bq_l = bq_full[cols]
        bk_l = bk_full[cols]
        bv_l = bv_full[cols]

        wqk_s = np.empty((2 * NP, 128, NE, 128), dtype=np.float16)
        for p in range(NP):
            wqk_s[2 * p] = wq_l[:, p * 128:(p + 1) * 128].reshape(NE, 128, 128).transpose(1, 0, 2)
            wqk_s[2 * p + 1] = wk_l[:, p * 128:(p + 1) * 128].reshape(NE, 128, 128).transpose(1, 0, 2)

        wv2 = np.zeros((E, VW), dtype=np.float16)
        bv2 = np.zeros((1, VW), dtype=np.float16)
        for h in range(HL):
            wv2[:, h * 65:h * 65 + 64] = wv_l[:, h * 64:(h + 1) * 64].astype(np.float16)
            bv2[0, h * 65:h * 65 + 64] = bv_l[h * 64:(h + 1) * 64].astype(np.float16)
            bv2[0, h * 65 + 64] = 1.0

        bcol = np.zeros((128, 2 * NP), dtype=np.float32)
        for p in range(NP):
            bcol[:, 2 * p] = bq_l[p * 128:(p + 1) * 128]
            bcol[:, 2 * p + 1] = bk_l[p * 128:(p + 1) * 128]

        wv2d = wv2.reshape(NE, 128, 2, VW // 2).transpose(2, 1, 0, 3)
        in_maps.append({
            "xT": np.ascontiguousarray(x[b].T.astype(np.float16)),
            "wqk": np.ascontiguousarray(wqk_s),
            "wv2d": np.ascontiguousarray(wv2d),
            "wo": np.ascontiguousarray(w_out[g * EL:(g + 1) * EL, :]).astype(np.float16),
            "rowsd": bv2,
            "bcold": bcol,
            "trid": tri,
        })
    return in_maps


def gather_output(results, b_out):
    out = np.empty((B, T, E), dtype=np.float32)
    for b in range(B):
        out[b] = results[2 * b]["y"] + results[2 * b + 1]["y"] + b_out[None, :]
    return out


def kernel(x, w_qkv, b_qkv, w_out, b_out):
    from concourse.bass_utils import run_bass_kernel_spmd

    nc = get_nc()
    in_maps = make_in_maps(x, w_qkv, b_qkv, w_out, b_out)
    r = run_bass_kernel_spmd(nc, in_maps, core_ids=list(range(NCORES)))
    return gather_output(r.results, np.asarray(b_out, dtype=np.float32))
